# revision 32
# baseline (speedup 1.0000x reference)
"""BandSplit kernel for Trainium2 (8 NeuronCores, batch-parallel).

Math (per band i with offset off, width b, K = 2b):
  x[t,k]   : band slice of X, k ordered (c,f) = re-plane rows then im-plane
  z = ((x-mu)*rsqrt(var+eps)*gamma + beta) @ W + bias
    = rsqrt[t] * ( x @ Wg  +  mu[t]*(-colsum)  +  sigma[t]*cvec )
  with Wg = gamma*W (rows), colsum = sum_k Wg[k,:], cvec = beta@W + bias[i],
  sigma = sqrt(var+eps), rsqrt = 1/sigma.

Each output tile [128t, 512d] is ONE accumulation group of fp16 matmuls
(lhsT = k-major x rows + a mu row + a sigma row, rhs = augmented W) followed
by a per-partition rsqrt scale fused into the PSUM->SBUF copy (fp16 out).

Host side: X is pre-permuted to the (c,f) k-row layout and cast to fp16,
so the device never deinterleaves; W_aug is packed fp16. Output staged
fp16 (halves the dominant store traffic), cast back to f32 on host.
Stats (mean / mean-of-squares) are computed by tiny PE matmuls against a
1/(2b)-valued rhs, batched 4 bands per PSUM tile for the element-wise
stats pipeline. Per core: batch element = core index. No collectives.
"""
import sys

sys.path.insert(0, "/opt/trn_rl_repo")
import numpy as np

BAND_BINS = [8] * 8 + [16] * 8 + [32] * 8 + [64] * 4 + [128] * 2 + [65]
NB = len(BAND_BINS)  # 31
D = 512
T = 1024
F = sum(BAND_BINS)  # 1025
EPS = 1e-5
NCORES = 8
NJ = T // 128  # 8 t-chunks
GSZ = 4  # bands per stats group
PIPE = 5  # band-back pipeline depth
LOOK = 2  # load prefetch distance, in groups


def plan():
    """Per-band chunk decomposition. Chunk rows: [x-rows ..., mu, sigma] where
    only the LAST chunk of a band carries the mu/sigma rows. x-rows of chunk c
    are rows [xq, xq+rows_x) of the host-packed Xp matrix; the chunk occupies
    rows [wrow0, wrow0+rows) of W_aug."""
    bands = []
    off = 0
    wrow = 0
    xq = 0
    for b in BAND_BINS:
        chunks = []
        if 2 * b + 2 <= 128:
            chunks.append(dict(rows_x=2 * b, has_ms=True, wrow0=wrow, xq=xq))
            wrow += 2 * b + 2
            xq += 2 * b
        else:
            # split at the re/im plane boundary
            chunks.append(dict(rows_x=b, has_ms=False, wrow0=wrow, xq=xq))
            wrow += b
            xq += b
            if b + 2 <= 128:
                chunks.append(dict(rows_x=b, has_ms=True, wrow0=wrow, xq=xq))
                wrow += b + 2
                xq += b
            else:
                chunks.append(dict(rows_x=b, has_ms=False, wrow0=wrow, xq=xq))
                wrow += b
                xq += b
                chunks.append(dict(rows_x=0, has_ms=True, wrow0=wrow, xq=xq))
                wrow += 2
        bands.append(dict(off=off, b=b, chunks=chunks))
        off += b
    return bands, wrow, xq


BANDS, W_ROWS, X_ROWS = plan()  # W_ROWS == 2112, X_ROWS == 2050


def build_w_aug(gamma, beta, W, bias):
    """Host-side: augmented, per-band-(c,f)-reordered weight matrix, fp16.
    k-order inside a band: r = c*b + f (re plane rows then im plane rows)."""
    w_aug = np.zeros((W_ROWS, D), dtype=np.float32)
    wg = gamma[:, None] * W  # [2F, D]
    for i, bd in enumerate(BANDS):
        off, b = bd["off"], bd["b"]
        s2 = 2 * off
        kidx = np.empty(2 * b, dtype=np.int64)
        kidx[0:b] = s2 + 2 * np.arange(b)          # re rows (c=0)
        kidx[b:2 * b] = s2 + 2 * np.arange(b) + 1  # im rows (c=1)
        xw = wg[kidx]  # [2b, D]
        colsum = xw.sum(axis=0)
        cvec = beta[s2:s2 + 2 * b] @ W[s2:s2 + 2 * b] + bias[i]
        r0 = 0
        for ch in bd["chunks"]:
            rx = ch["rows_x"]
            w_aug[ch["wrow0"]:ch["wrow0"] + rx] = xw[r0:r0 + rx]
            r0 += rx
            if ch["has_ms"]:
                w_aug[ch["wrow0"] + rx] = -colsum
                w_aug[ch["wrow0"] + rx + 1] = cvec
    return w_aug.astype(np.float16)


_XPERM = None


def build_xp(Xb):
    """Host-side: [F, T, 2] f32 -> [2050, T] fp16, rows in per-band (c,f)
    order (matches W_aug x-rows)."""
    global _XPERM
    if _XPERM is None:
        parts = []
        for bd in BANDS:
            off, b = bd["off"], bd["b"]
            parts.append(np.arange(off, off + b))       # re rows
            parts.append(F + np.arange(off, off + b))   # im rows
        _XPERM = np.concatenate(parts)
    src = np.concatenate([Xb[:, :, 0], Xb[:, :, 1]], axis=0)  # [2F, T]
    return np.ascontiguousarray(src[_XPERM]).astype(np.float16)


def build_nc():
    import concourse.bacc as bacc
    import concourse.tile as tile
    from concourse import mybir
    from concourse.masks import make_identity

    f32, f16 = mybir.dt.float32, mybir.dt.float16
    Sqrt = mybir.ActivationFunctionType.Sqrt
    Copy = mybir.ActivationFunctionType.Copy

    nc = bacc.Bacc(None)
    XP = nc.declare_dram_parameter("XP", [X_ROWS, T], f16, isOutput=False)
    WA = nc.declare_dram_parameter("WA", [W_ROWS, D], f16, isOutput=False)
    OUT = nc.declare_dram_parameter("OUT", [NB, T, D], f16, isOutput=True)

    groups = [list(range(g, min(g + GSZ, NB))) for g in range(0, NB, GSZ)]

    with tile.TileContext(nc) as tc:
        with tc.tile_pool(name="consts", bufs=1) as consts, \
             tc.tile_pool(name="kx", bufs=30) as kxp, \
             tc.tile_pool(name="x2", bufs=8) as x2p, \
             tc.tile_pool(name="wp", bufs=30) as wp, \
             tc.tile_pool(name="stat", bufs=3) as statp, \
             tc.tile_pool(name="stage", bufs=4) as stagep, \
             tc.tile_pool(name="pso", bufs=6, space="PSUM") as psop, \
             tc.tile_pool(name="pss", bufs=2, space="PSUM") as pssp:

            ident = consts.tile([128, 128], f32, tag="ident")
            epsc = consts.tile([128, 1], f32, tag="epsc")
            invs = {}

            def emit_consts():
                # emitted after the first load issues: make_identity's Pool
                # ops must not delay the first SWDGE load generations
                make_identity(nc, ident)
                nc.vector.memset(epsc, EPS)
                # per-distinct-width 1/(2b) stats rhs (fp16, matches kx)
                for b in sorted(set(BAND_BINS)):
                    it = consts.tile([128, 2], f16, tag=f"inv{b}",
                                     name=f"inv{b}")
                    nc.vector.memset(it, 1.0 / (2 * b))
                    invs[b] = it

            # ---- just-in-time prefetch: kx x-rows on SWDGE (Pool), W on the
            # SP HWDGE queue; issued LOOK groups ahead of the stats front so
            # the first group's deps land within ~5us of t=0.
            kxss = [None] * NB
            wtss = [None] * NB

            def issue_kx(i, kx_eng):
                kxs = []
                for ch in BANDS[i]["chunks"]:
                    rows = ch["rows_x"] + (2 if ch["has_ms"] else 0)
                    kx = kxp.tile([rows, T], f16, tag="kx", name="kx")
                    if ch["rows_x"] > 0:
                        kx_eng.dma_start(
                            out=kx[0:ch["rows_x"], :],
                            in_=XP[ch["xq"]:ch["xq"] + ch["rows_x"], :])
                    kxs.append(kx)
                kxss[i] = kxs

            def issue_wt(i):
                wts = []
                for ch in BANDS[i]["chunks"]:
                    rows = ch["rows_x"] + (2 if ch["has_ms"] else 0)
                    wt = wp.tile([rows, D], f16, tag="W", name="wt")
                    nc.sync.dma_start(
                        out=wt, in_=WA[ch["wrow0"]:ch["wrow0"] + rows, :])
                    wts.append(wt)
                wtss[i] = wts

            def issue_loads(gi):
                for i in groups[gi]:
                    issue_kx(i, nc.gpsimd)
                    issue_wt(i)

            eng_flip = [0]

            def flip():
                eng_flip[0] += 1
                return eng_flip[0] % 2 == 0

            def emit_front1(gi):
                """squares + stats matmuls + batched element-wise stats
                pipeline (through rsqrt) for one group of bands."""
                bids = groups[gi]
                g_n = len(bids)
                # one PSUM bank per group: cols [0,128) stats accum,
                # [128,256) transpose target (partitions 0..16*g_n)
                pcmt = pssp.tile([128, 512], f32, tag="pcmt", name="pcmt")
                pc4 = pcmt[:, 0:32 * g_n]
                for g, i in enumerate(bids):
                    bd = BANDS[i]
                    b = bd["b"]
                    chunks = bd["chunks"]
                    kxs = kxss[i]
                    x_chunks = [(ci, ch) for ci, ch in enumerate(chunks)
                                if ch["rows_x"] > 0]
                    last_x = len(x_chunks) - 1
                    x2s = []
                    for (ci, ch) in x_chunks:
                        rx = ch["rows_x"]
                        x2 = x2p.tile([rx, T], f16, tag="x2")
                        if flip():
                            nc.vector.tensor_mul(x2, kxs[ci][0:rx, :],
                                                 kxs[ci][0:rx, :])
                        else:
                            nc.scalar.activation(
                                out=x2, in_=kxs[ci][0:rx, :],
                                func=mybir.ActivationFunctionType.Square)
                        x2s.append(x2)
                    inv = invs[b]
                    for j in range(NJ):
                        for xi, (ci, ch) in enumerate(x_chunks):
                            rx = ch["rows_x"]
                            nc.tensor.matmul(
                                pc4[:, 32 * g + 2 * j:32 * g + 2 * j + 2],
                                kxs[ci][0:rx, j * 128:(j + 1) * 128],
                                inv[0:rx, :],
                                start=(xi == 0), stop=(xi == last_x))
                    for j in range(NJ):
                        for xi, (ci, ch) in enumerate(x_chunks):
                            rx = ch["rows_x"]
                            nc.tensor.matmul(
                                pc4[:, 32 * g + 16 + 2 * j:32 * g + 18 + 2 * j],
                                x2s[xi][:, j * 128:(j + 1) * 128],
                                inv[0:rx, :],
                                start=(xi == 0), stop=(xi == last_x))

                # batched stats pipeline over the whole group
                # pc4 col = g*32 + s*16 + 2a + c  (s: 0=mean,1=mean sq; c dup)
                nw = g_n * 8
                mu = pc4[:, :].rearrange("p (g s a c) -> p s c g a",
                                         s=2, a=NJ, c=2)[:, 0, 0]   # [128,g,8]
                ex2 = pc4[:, :].rearrange("p (g s a c) -> p s c g a",
                                          s=2, a=NJ, c=2)[:, 1, 0]
                ms4 = statp.tile([128, 16 * g_n], f32, tag="ms4")
                msv = ms4[:, :].rearrange("p (g s a) -> p s g a", s=2, a=NJ)
                t1 = statp.tile([128, nw], f32, tag="t1")
                t1v = t1[:, :].rearrange("p (g a) -> p g a", a=NJ)
                rs4 = statp.tile([128, nw], f32, tag="rs4")
                rs4v = rs4[:, :].rearrange("p (g a) -> p g a", a=NJ)
                # (hw: at most one PSUM input per op -> mu lands in SBUF
                # ms4 first, then gets squared from there)
                nc.scalar.activation(out=msv[:, 0], in_=mu, func=Copy)
                nc.vector.tensor_mul(t1v, msv[:, 0], msv[:, 0])  # mu^2
                nc.vector.tensor_sub(t1v, ex2, t1v)              # var
                nc.scalar.activation(out=msv[:, 1], in_=t1v, func=Sqrt,
                                     bias=epsc, scale=1.0)       # sigma
                nc.vector.reciprocal(out=rs4v, in_=msv[:, 1])    # rsqrt
                return (bids, g_n, pcmt, ms4, rs4)

            def emit_front2(st):
                """mu/sigma rows via PE transpose + per-band partition-fold
                DMA. Emitted AFTER a back-drain so the transpose's wait on
                ms4 doesn't head-of-line-block back matmuls in PE.SEQ."""
                bids, g_n, pcmt, ms4, rs4 = st
                mt4 = pcmt[0:16 * g_n, 128:256]
                nc.tensor.transpose(mt4, ms4, ident)
                mts4 = statp.tile([16 * g_n, 128], f16, tag="mts4")
                nc.vector.tensor_scalar_mul(mts4, mt4, 1.0)
                for g, i in enumerate(bids):
                    chunks = BANDS[i]["chunks"]
                    rem = chunks[-1]["rows_x"]
                    kxl = kxss[i][-1]
                    nc.scalar.dma_start(
                        out=kxl[rem:rem + 2, :].rearrange(
                            "r (j p) -> r j p", j=NJ),
                        in_=mts4[16 * g:16 * g + 16, :])
                return [(i, rs4, 8 * bids.index(i)) for i in bids]

            def emit_back(i, rs4, rcol):
                """main matmuls + scale-copy + out DMA for band i"""
                kxs, wts = kxss[i], wtss[i]
                chunks = BANDS[i]["chunks"]
                stage = stagep.tile([128, NJ, D], f16, tag="stage")
                for j in range(NJ):
                    po = psop.tile([128, D], f32, tag="po")
                    for ci, ch in enumerate(chunks):
                        rows = ch["rows_x"] + (2 if ch["has_ms"] else 0)
                        nc.tensor.matmul(
                            po, kxs[ci][0:rows, j * 128:(j + 1) * 128],
                            wts[ci][0:rows, :],
                            start=(ci == 0), stop=(ci == len(chunks) - 1))
                    sc = rs4[:, rcol + j:rcol + j + 1]
                    if flip():
                        nc.vector.tensor_scalar_mul(stage[:, j, :], po, sc)
                    else:
                        nc.scalar.activation(out=stage[:, j, :], in_=po,
                                             func=Copy, scale=sc)
                nc.sync.dma_start(
                    out=OUT[i, :, :].rearrange("(j p) d -> p j d", p=128),
                    in_=stage)

            # ---- software pipeline: stats-front groups run ahead of the
            # per-band back stage by ~PIPE bands; loads LOOK groups ahead.
            from collections import deque
            ngr = len(groups)
            pend = deque()
            for gi in range(ngr):
                if gi == 0:
                    # prologue: kx for the first LOOK+1 groups round-robin
                    # across all three DMA-issue queues (3x the gen rate of
                    # serial Pool SWDGE), W afterwards (needed ~8us later)
                    engs = [nc.gpsimd, nc.sync, nc.scalar]
                    k = 0
                    pro = range(min(LOOK + 1, ngr))
                    for g2 in pro:
                        for i in groups[g2]:
                            issue_kx(i, engs[k % 3])
                            k += 1
                    emit_consts()
                    for g2 in pro:
                        for i in groups[g2]:
                            issue_wt(i)
                elif gi + LOOK < ngr:
                    issue_loads(gi + LOOK)
                st = emit_front1(gi)
                # taper the backlog near the end so the tail isn't a long
                # serial back-drain after the last front
                thr = PIPE if gi < ngr - 2 else 2
                while len(pend) > thr:
                    emit_back(*pend.popleft())
                pend.extend(emit_front2(st))
            while pend:
                emit_back(*pend.popleft())

    nc.finalize()
    return nc


_NC = None


def prepare_in_maps(X, gamma, beta, W, bias):
    w_aug = build_w_aug(gamma, beta, W, bias)
    return [{"XP": build_xp(X[b]), "WA": w_aug} for b in range(NCORES)]


def kernel(X, gamma, beta, W, bias):
    global _NC
    from concourse.bass_utils import run_bass_kernel_spmd

    X = np.asarray(X, dtype=np.float32)
    gamma = np.asarray(gamma, dtype=np.float32)
    beta = np.asarray(beta, dtype=np.float32)
    W = np.asarray(W, dtype=np.float32)
    bias = np.asarray(bias, dtype=np.float32)

    in_maps = prepare_in_maps(X, gamma, beta, W, bias)
    if _NC is None:
        _NC = build_nc()
    res = run_bass_kernel_spmd(_NC, in_maps, list(range(NCORES))).results
    return np.stack([res[b]["OUT"] for b in range(NCORES)],
                    axis=0).astype(np.float32)


# revision 33
# speedup vs baseline: 1.0212x; 1.0212x over previous
"""BandSplit kernel for Trainium2 (8 NeuronCores, batch-parallel).

Math (per band i with offset off, width b, K = 2b):
  x[t,k]   : band slice of X, k ordered (c,f) = re-plane rows then im-plane
  z = ((x-mu)*rsqrt(var+eps)*gamma + beta) @ W + bias
    = rsqrt[t] * ( x @ Wg  +  mu[t]*(-colsum)  +  sigma[t]*cvec )
  with Wg = gamma*W (rows), colsum = sum_k Wg[k,:], cvec = beta@W + bias[i],
  sigma = sqrt(var+eps), rsqrt = 1/sigma.

Each output tile [128t, 512d] is ONE accumulation group of fp16 matmuls
(lhsT = k-major x rows + a mu row + a sigma row, rhs = augmented W) followed
by a per-partition rsqrt scale fused into the PSUM->SBUF copy (fp16 out).

Host side: X is pre-permuted to the (c,f) k-row layout and cast to fp16,
so the device never deinterleaves; W_aug is packed fp16. Output staged
fp16 (halves the dominant store traffic), cast back to f32 on host.
Stats (mean / mean-of-squares) are computed by tiny PE matmuls against a
1/(2b)-valued rhs, batched 4 bands per PSUM tile for the element-wise
stats pipeline. Per core: batch element = core index. No collectives.
"""
import sys

sys.path.insert(0, "/opt/trn_rl_repo")
import numpy as np

BAND_BINS = [8] * 8 + [16] * 8 + [32] * 8 + [64] * 4 + [128] * 2 + [65]
NB = len(BAND_BINS)  # 31
D = 512
T = 1024
F = sum(BAND_BINS)  # 1025
EPS = 1e-5
NCORES = 8
NJ = T // 128  # 8 t-chunks
GSZ = 4  # bands per stats group
PIPE = 5  # band-back pipeline depth
LOOK = 2  # load prefetch distance, in groups


def plan():
    """Per-band chunk decomposition. Chunk rows: [x-rows ..., mu, sigma] where
    only the LAST chunk of a band carries the mu/sigma rows. x-rows of chunk c
    are rows [xq, xq+rows_x) of the host-packed Xp matrix; the chunk occupies
    rows [wrow0, wrow0+rows) of W_aug."""
    bands = []
    off = 0
    wrow = 0
    xq = 0
    for b in BAND_BINS:
        chunks = []
        if 2 * b + 2 <= 128:
            chunks.append(dict(rows_x=2 * b, has_ms=True, wrow0=wrow, xq=xq))
            wrow += 2 * b + 2
            xq += 2 * b
        else:
            # split at the re/im plane boundary
            chunks.append(dict(rows_x=b, has_ms=False, wrow0=wrow, xq=xq))
            wrow += b
            xq += b
            if b + 2 <= 128:
                chunks.append(dict(rows_x=b, has_ms=True, wrow0=wrow, xq=xq))
                wrow += b + 2
                xq += b
            else:
                chunks.append(dict(rows_x=b, has_ms=False, wrow0=wrow, xq=xq))
                wrow += b
                xq += b
                chunks.append(dict(rows_x=0, has_ms=True, wrow0=wrow, xq=xq))
                wrow += 2
        bands.append(dict(off=off, b=b, chunks=chunks))
        off += b
    return bands, wrow, xq


BANDS, W_ROWS, X_ROWS = plan()  # W_ROWS == 2112, X_ROWS == 2050


def build_w_aug(gamma, beta, W, bias):
    """Host-side: augmented, per-band-(c,f)-reordered weight matrix, fp16.
    k-order inside a band: r = c*b + f (re plane rows then im plane rows)."""
    w_aug = np.zeros((W_ROWS, D), dtype=np.float32)
    wg = gamma[:, None] * W  # [2F, D]
    for i, bd in enumerate(BANDS):
        off, b = bd["off"], bd["b"]
        s2 = 2 * off
        kidx = np.empty(2 * b, dtype=np.int64)
        kidx[0:b] = s2 + 2 * np.arange(b)          # re rows (c=0)
        kidx[b:2 * b] = s2 + 2 * np.arange(b) + 1  # im rows (c=1)
        xw = wg[kidx]  # [2b, D]
        colsum = xw.sum(axis=0)
        cvec = beta[s2:s2 + 2 * b] @ W[s2:s2 + 2 * b] + bias[i]
        r0 = 0
        for ch in bd["chunks"]:
            rx = ch["rows_x"]
            w_aug[ch["wrow0"]:ch["wrow0"] + rx] = xw[r0:r0 + rx]
            r0 += rx
            if ch["has_ms"]:
                w_aug[ch["wrow0"] + rx] = -colsum
                w_aug[ch["wrow0"] + rx + 1] = cvec
    return w_aug.astype(np.float16)


_XPERM = None


def build_xp(Xb):
    """Host-side: [F, T, 2] f32 -> [2050, T] fp16, rows in per-band (c,f)
    order (matches W_aug x-rows)."""
    global _XPERM
    if _XPERM is None:
        parts = []
        for bd in BANDS:
            off, b = bd["off"], bd["b"]
            parts.append(np.arange(off, off + b))       # re rows
            parts.append(F + np.arange(off, off + b))   # im rows
        _XPERM = np.concatenate(parts)
    src = np.concatenate([Xb[:, :, 0], Xb[:, :, 1]], axis=0)  # [2F, T]
    return np.ascontiguousarray(src[_XPERM]).astype(np.float16)


def build_nc():
    import concourse.bacc as bacc
    import concourse.tile as tile
    from concourse import mybir
    from concourse.masks import make_identity

    f32, f16 = mybir.dt.float32, mybir.dt.float16
    Sqrt = mybir.ActivationFunctionType.Sqrt
    Copy = mybir.ActivationFunctionType.Copy

    nc = bacc.Bacc(None)
    XP = nc.declare_dram_parameter("XP", [X_ROWS, T], f16, isOutput=False)
    WA = nc.declare_dram_parameter("WA", [W_ROWS, D], f16, isOutput=False)
    OUT = nc.declare_dram_parameter("OUT", [NB, T, D], f16, isOutput=True)

    groups = [list(range(g, min(g + GSZ, NB))) for g in range(0, NB, GSZ)]

    with tile.TileContext(nc) as tc:
        with tc.tile_pool(name="consts", bufs=1) as consts, \
             tc.tile_pool(name="kx", bufs=30) as kxp, \
             tc.tile_pool(name="x2", bufs=8) as x2p, \
             tc.tile_pool(name="wp", bufs=30) as wp, \
             tc.tile_pool(name="stat", bufs=3) as statp, \
             tc.tile_pool(name="stage", bufs=4) as stagep, \
             tc.tile_pool(name="pso", bufs=6, space="PSUM") as psop, \
             tc.tile_pool(name="pss", bufs=2, space="PSUM") as pssp:

            ident = consts.tile([128, 128], f32, tag="ident")
            epsc = consts.tile([128, 1], f32, tag="epsc")
            invs = {}

            def emit_consts():
                # emitted after the first load issues: make_identity's Pool
                # ops must not delay the first SWDGE load generations
                make_identity(nc, ident)
                nc.vector.memset(epsc, EPS)
                # per-distinct-width 1/(2b) stats rhs (fp16, matches kx)
                for b in sorted(set(BAND_BINS)):
                    it = consts.tile([128, 2], f16, tag=f"inv{b}",
                                     name=f"inv{b}")
                    nc.vector.memset(it, 1.0 / (2 * b))
                    invs[b] = it

            # ---- just-in-time prefetch: kx x-rows on SWDGE (Pool), W on the
            # SP HWDGE queue; issued LOOK groups ahead of the stats front so
            # the first group's deps land within ~5us of t=0.
            kxss = [None] * NB
            wtss = [None] * NB

            def issue_kx(i, kx_eng):
                kxs = []
                for ch in BANDS[i]["chunks"]:
                    rows = ch["rows_x"] + (2 if ch["has_ms"] else 0)
                    kx = kxp.tile([rows, T], f16, tag="kx", name="kx")
                    if ch["rows_x"] > 0:
                        kx_eng.dma_start(
                            out=kx[0:ch["rows_x"], :],
                            in_=XP[ch["xq"]:ch["xq"] + ch["rows_x"], :])
                    kxs.append(kx)
                kxss[i] = kxs

            def issue_wt(i):
                wts = []
                for ch in BANDS[i]["chunks"]:
                    rows = ch["rows_x"] + (2 if ch["has_ms"] else 0)
                    wt = wp.tile([rows, D], f16, tag="W", name="wt")
                    nc.sync.dma_start(
                        out=wt, in_=WA[ch["wrow0"]:ch["wrow0"] + rows, :])
                    wts.append(wt)
                wtss[i] = wts

            def issue_loads(gi):
                for i in groups[gi]:
                    issue_kx(i, nc.gpsimd)
                    issue_wt(i)

            eng_flip = [0]

            def flip():
                eng_flip[0] += 1
                return eng_flip[0] % 2 == 0

            def emit_front1(gi):
                """squares + stats matmuls + batched element-wise stats
                pipeline (through rsqrt) for one group of bands."""
                bids = groups[gi]
                g_n = len(bids)
                # one PSUM bank per group: cols [0,128) stats accum,
                # [128,256) transpose target (partitions 0..16*g_n)
                pcmt = pssp.tile([128, 512], f32, tag="pcmt", name="pcmt")
                pc4 = pcmt[:, 0:32 * g_n]
                for g, i in enumerate(bids):
                    bd = BANDS[i]
                    b = bd["b"]
                    chunks = bd["chunks"]
                    kxs = kxss[i]
                    x_chunks = [(ci, ch) for ci, ch in enumerate(chunks)
                                if ch["rows_x"] > 0]
                    last_x = len(x_chunks) - 1
                    x2s = []
                    for (ci, ch) in x_chunks:
                        rx = ch["rows_x"]
                        x2 = x2p.tile([rx, T], f16, tag="x2")
                        if flip():
                            nc.vector.tensor_mul(x2, kxs[ci][0:rx, :],
                                                 kxs[ci][0:rx, :])
                        else:
                            nc.scalar.activation(
                                out=x2, in_=kxs[ci][0:rx, :],
                                func=mybir.ActivationFunctionType.Square)
                        x2s.append(x2)
                    inv = invs[b]
                    for j in range(NJ):
                        for xi, (ci, ch) in enumerate(x_chunks):
                            rx = ch["rows_x"]
                            nc.tensor.matmul(
                                pc4[:, 32 * g + 2 * j:32 * g + 2 * j + 2],
                                kxs[ci][0:rx, j * 128:(j + 1) * 128],
                                inv[0:rx, :],
                                start=(xi == 0), stop=(xi == last_x))
                    for j in range(NJ):
                        for xi, (ci, ch) in enumerate(x_chunks):
                            rx = ch["rows_x"]
                            nc.tensor.matmul(
                                pc4[:, 32 * g + 16 + 2 * j:32 * g + 18 + 2 * j],
                                x2s[xi][:, j * 128:(j + 1) * 128],
                                inv[0:rx, :],
                                start=(xi == 0), stop=(xi == last_x))

                # batched stats pipeline over the whole group
                # pc4 col = g*32 + s*16 + 2a + c  (s: 0=mean,1=mean sq; c dup)
                nw = g_n * 8
                mu = pc4[:, :].rearrange("p (g s a c) -> p s c g a",
                                         s=2, a=NJ, c=2)[:, 0, 0]   # [128,g,8]
                ex2 = pc4[:, :].rearrange("p (g s a c) -> p s c g a",
                                          s=2, a=NJ, c=2)[:, 1, 0]
                ms4 = statp.tile([128, 16 * g_n], f32, tag="ms4")
                msv = ms4[:, :].rearrange("p (g s a) -> p s g a", s=2, a=NJ)
                t1 = statp.tile([128, nw], f32, tag="t1")
                t1v = t1[:, :].rearrange("p (g a) -> p g a", a=NJ)
                rs4 = statp.tile([128, nw], f32, tag="rs4")
                rs4v = rs4[:, :].rearrange("p (g a) -> p g a", a=NJ)
                # (hw: at most one PSUM input per op -> mu lands in SBUF
                # ms4 first, then gets squared from there)
                nc.scalar.activation(out=msv[:, 0], in_=mu, func=Copy)
                nc.vector.tensor_mul(t1v, msv[:, 0], msv[:, 0])  # mu^2
                nc.vector.tensor_sub(t1v, ex2, t1v)              # var
                nc.scalar.activation(out=msv[:, 1], in_=t1v, func=Sqrt,
                                     bias=epsc, scale=1.0)       # sigma
                nc.vector.reciprocal(out=rs4v, in_=msv[:, 1])    # rsqrt
                return (bids, g_n, pcmt, ms4, rs4)

            def emit_front2(st):
                """mu/sigma rows via PE transpose + per-band partition-fold
                DMA. Emitted AFTER a back-drain so the transpose's wait on
                ms4 doesn't head-of-line-block back matmuls in PE.SEQ."""
                bids, g_n, pcmt, ms4, rs4 = st
                mt4 = pcmt[0:16 * g_n, 128:256]
                nc.tensor.transpose(mt4, ms4, ident)
                mts4 = statp.tile([16 * g_n, 128], f16, tag="mts4")
                nc.vector.tensor_scalar_mul(mts4, mt4, 1.0)
                for g, i in enumerate(bids):
                    chunks = BANDS[i]["chunks"]
                    rem = chunks[-1]["rows_x"]
                    kxl = kxss[i][-1]
                    nc.scalar.dma_start(
                        out=kxl[rem:rem + 2, :].rearrange(
                            "r (j p) -> r j p", j=NJ),
                        in_=mts4[16 * g:16 * g + 16, :])
                return [(i, rs4, 8 * bids.index(i)) for i in bids]

            def emit_back(i, rs4, rcol):
                """main matmuls + scale-copy + out DMA for band i"""
                kxs, wts = kxss[i], wtss[i]
                chunks = BANDS[i]["chunks"]
                stage = stagep.tile([128, NJ, D], f16, tag="stage")
                for j in range(NJ):
                    po = psop.tile([128, D], f32, tag="po")
                    for ci, ch in enumerate(chunks):
                        rows = ch["rows_x"] + (2 if ch["has_ms"] else 0)
                        nc.tensor.matmul(
                            po, kxs[ci][0:rows, j * 128:(j + 1) * 128],
                            wts[ci][0:rows, :],
                            start=(ci == 0), stop=(ci == len(chunks) - 1))
                    sc = rs4[:, rcol + j:rcol + j + 1]
                    if flip():
                        nc.vector.tensor_scalar_mul(stage[:, j, :], po, sc)
                    else:
                        nc.scalar.activation(out=stage[:, j, :], in_=po,
                                             func=Copy, scale=sc)
                nc.sync.dma_start(
                    out=OUT[i, :, :].rearrange("(j p) d -> p j d", p=128),
                    in_=stage)

            # ---- software pipeline: stats-front groups run ahead of the
            # per-band back stage by ~PIPE bands; loads LOOK groups ahead.
            from collections import deque
            ngr = len(groups)
            pend = deque()
            for gi in range(ngr):
                if gi == 0:
                    issue_loads(0)
                    emit_consts()
                    for g2 in range(1, min(LOOK + 1, ngr)):
                        issue_loads(g2)
                elif gi + LOOK < ngr:
                    issue_loads(gi + LOOK)
                st = emit_front1(gi)
                # taper the backlog near the end so the tail isn't a long
                # serial back-drain after the last front
                thr = PIPE if gi < ngr - 2 else 2
                while len(pend) > thr:
                    emit_back(*pend.popleft())
                pend.extend(emit_front2(st))
            while pend:
                emit_back(*pend.popleft())

    nc.finalize()
    return nc


_NC = None


def prepare_in_maps(X, gamma, beta, W, bias):
    w_aug = build_w_aug(gamma, beta, W, bias)
    return [{"XP": build_xp(X[b]), "WA": w_aug} for b in range(NCORES)]


def kernel(X, gamma, beta, W, bias):
    global _NC
    from concourse.bass_utils import run_bass_kernel_spmd

    X = np.asarray(X, dtype=np.float32)
    gamma = np.asarray(gamma, dtype=np.float32)
    beta = np.asarray(beta, dtype=np.float32)
    W = np.asarray(W, dtype=np.float32)
    bias = np.asarray(bias, dtype=np.float32)

    in_maps = prepare_in_maps(X, gamma, beta, W, bias)
    if _NC is None:
        _NC = build_nc()
    res = run_bass_kernel_spmd(_NC, in_maps, list(range(NCORES))).results
    return np.stack([res[b]["OUT"] for b in range(NCORES)],
                    axis=0).astype(np.float32)
